# revision 75
# baseline (speedup 1.0000x reference)
"""Trainium2 Bass kernel for 12-head causal MHA (B=2, S=2048, D=768), fp32 I/O.

Sharding: 8 cores = (batch b in {0,1}) x (head-group hg in {0..3}, 3 heads each).
Each core computes, for its (b, hg):
    qT/kT = (x wq_hg^T)^T  (transposed layout, [192, S], bf16)
    v     = x wv_hg^T      ([S, 192] natural + ones column, bf16)
    causal attention with et (=exp(scores)) kept per block in SBUF, then a
    narrow-N attnv: out[128q, 65] = et[keys, q]^T @ [v | 1] accumulated over
    key tiles (bf16 runs at 1 cyc/row at ANY N, so N=65 is cheap), softmax
    denominator lands per-partition -> reciprocal + tensor_scalar_mul.
    ot[128q, 64] is PE-transposed back to [64, 128q] for the output proj.
    partial yT = wo_hg^T @ outT   ([768, S] bf16, row-parallel partial)
Host sums the 4 head-group partials per batch, transposes, adds bo.

All matmuls bf16 (1 cycle/row at any moving size). Exp on the Activation
engine with full score tiles paired into [128, 2*512] psum tiles (one exp per
pair amortizes the ~185ns Act instruction overhead). DVE handles bias adds,
normalize, and psum->sbuf drains (GPSIMD cannot touch PSUM); the all-SBUF
causal-mask multiplies run on GpSimd. Emission is software-pipelined: block
j's scores stream interleaves block j-1's attnv/output-proj plus block j+1's
projections as PE filler while Act drains exps; the last block's attnv chains
ride inside its own stream (head h's chains fill head h+1's score stream).
"""

import math
from contextlib import ExitStack

import numpy as np

import concourse.bacc as bacc
import concourse.bass as bass
import concourse.mybir as mybir
import concourse.tile as tile

FP32 = mybir.dt.float32
BF16 = mybir.dt.bfloat16

B = 2
S = 2048
D = 768
NH = 12
DK = 64
NCORES = 8
HG = 3  # heads per core
HD = HG * DK  # 192
KC = D // 128  # 6 contraction chunks of 128
SB = 512  # sequence block
NJ = S // SB  # 4
NT = S // 128  # 16 key tiles
SCALE = 1.0 / math.sqrt(DK)
VW = 65  # v tile width (64 dims + ones column for the softmax denominator)


def build_nc(causal: bool):
    nc = bacc.Bacc(trn_type="TRN2", target_bir_lowering=False, debug=False)

    xT_d = nc.declare_dram_parameter("xT", [D, S], BF16, isOutput=False)
    # packed weights (row-contiguous for fat DMA descriptors):
    # wqk01: cols 0:128 wq[0:128].T | 128:256 wk[0:128].T
    # wqk2v: cols 0:64 wq[128:192].T | 64:128 wk[128:192].T | 128:320 wv.T
    wqk01_d = nc.declare_dram_parameter("wqk01", [D, 256], BF16, isOutput=False)
    wqk2v_d = nc.declare_dram_parameter("wqk2v", [D, 320], BF16, isOutput=False)
    # wo packed: [:, 0:768] = wo[:, 0:128].T ; [0:64, 768:1536] = wo[:,128:192].T
    woTP_d = nc.declare_dram_parameter("woTP", [128, 2 * D], BF16, isOutput=False)
    bqk0_d = nc.declare_dram_parameter("bqk0", [128, 2], FP32, isOutput=False)
    bqk1_d = nc.declare_dram_parameter("bqk1", [128, 2], FP32, isOutput=False)
    bv_d = nc.declare_dram_parameter("bv", [HD], FP32, isOutput=False)
    # cols 0:128 = causal triangle mask (keep iff p <= c), 128:256 = identity
    cmid_d = nc.declare_dram_parameter("cmid", [128, 256], BF16, isOutput=False)
    yT_d = nc.declare_dram_parameter("yT", [D, S], BF16, isOutput=True)

    EXP = mybir.ActivationFunctionType.Exp

    with tile.TileContext(nc) as tc, ExitStack() as ctx:
        consts = ctx.enter_context(tc.tile_pool(name="consts", bufs=1))

        # ---- persistent SBUF tensors ----
        xT_sb = consts.tile([128, KC, S], BF16)
        wqk01_sb = consts.tile([128, KC, 256], BF16)
        wqk2v_sb = consts.tile([128, KC, 320], BF16)
        woTP_sb = consts.tile([128, 2 * D], BF16)
        bqk0_sb = consts.tile([128, 2], FP32)
        bqk1_sb = consts.tile([128, 2], FP32)
        bvb_sb = consts.tile([128, HG, 64], FP32)  # bv broadcast to partitions
        cmid_sb = consts.tile([128, 256], BF16)
        qT01_sb = consts.tile([128, S], BF16)  # q^T heads 0,1
        kT01_sb = consts.tile([128, S], BF16)
        qT2_sb = consts.tile([64, S], BF16)  # q^T head 2 (partitions 0:64)
        kT2_sb = consts.tile([64, S], BF16)  # k^T head 2 (DMA-hopped to 0:64)
        v_sb = consts.tile([128, NT, HG, VW], BF16)  # [v | ones]
        # et double-buffered by block parity (skewed pipeline overlaps blocks)
        et_sb = [consts.tile([128, NT, HG, SB], BF16, name=f"et{p}") for p in (0, 1)]
        otT01_sb = [consts.tile([128, SB], BF16, name=f"otT01_{p}") for p in (0, 1)]
        otT2_sb = [consts.tile([64, SB], BF16, name=f"otT2_{p}") for p in (0, 1)]

        cm_sb = cmid_sb[:, 0:128]
        id_sb = cmid_sb[:, 128:256]

        # ---- input DMAs: q0 weights first on SP (first matmul group),
        # x block 0 per-chunk on Pool, tiny constants on the DVE queue ----
        wqk01_r = wqk01_d.ap().rearrange("(c p) n -> p c n", p=128)
        wqk2v_r = wqk2v_d.ap().rearrange("(c p) n -> p c n", p=128)
        xT_r = xT_d.ap().rearrange("(c p) s -> p c s", p=128)
        nc.sync.dma_start(out=wqk01_sb, in_=wqk01_r)
        for c in range(KC // 2):
            nc.gpsimd.dma_start(out=xT_sb[:, c, 0:SB], in_=xT_r[:, c, 0:SB])
        nc.gpsimd.dma_start(out=bqk0_sb, in_=bqk0_d.ap())
        nc.gpsimd.dma_start(out=bqk1_sb, in_=bqk1_d.ap())
        nc.gpsimd.dma_start(out=cmid_sb, in_=cmid_d.ap())
        bv_ap = bv_d.ap()
        bvb_src = bass.AP(
            tensor=bv_ap.tensor, offset=bv_ap.offset, ap=[[0, 128], [64, HG], [1, 64]]
        )
        nc.gpsimd.dma_start(out=bvb_sb, in_=bvb_src)
        for c in range(KC // 2, KC):
            nc.scalar.dma_start(out=xT_sb[:, c, 0:SB], in_=xT_r[:, c, 0:SB])
        nc.sync.dma_start(out=wqk2v_sb, in_=wqk2v_r)
        nc.gpsimd.dma_start(out=xT_sb[:, :, SB : 2 * SB], in_=xT_r[:, :, SB : 2 * SB])
        nc.sync.dma_start(out=woTP_sb, in_=woTP_d.ap())
        for jb in range(2, NJ):
            nc.gpsimd.dma_start(
                out=xT_sb[:, :, jb * SB : (jb + 1) * SB],
                in_=xT_r[:, :, jb * SB : (jb + 1) * SB],
            )

        # preset the ones columns of v
        nc.vector.memset(v_sb[:, :, :, 64:65], 1.0)

        # ---- pools (PSUM: mix 2 + sp 4 + op 2 = 8 banks) ----
        mix_pool = ctx.enter_context(tc.tile_pool(name="mix", bufs=2, space="PSUM"))
        sp_pool = ctx.enter_context(tc.tile_pool(name="sp", bufs=2, space="PSUM"))
        op_pool = ctx.enter_context(tc.tile_pool(name="op", bufs=2, space="PSUM"))
        kt_pool = ctx.enter_context(tc.tile_pool(name="kt", bufs=3))
        rc_pool = ctx.enter_context(tc.tile_pool(name="rc", bufs=8))
        ot_pool = ctx.enter_context(tc.tile_pool(name="ot", bufs=14))
        yt_pool = ctx.enter_context(tc.tile_pool(name="yt", bufs=6))

        def tend(j):
            return 4 * (j + 1) if causal else NT

        # ---------- emission helpers ----------
        # Each filler is (pe_cost_ns, closure); streams interleave fillers
        # between score units proportionally to accumulated PE cost so the
        # Activation engine is never starved of scores nor left to lag.
        def v_proj_tile(st):
            def emit():
                vp = mix_pool.tile([128, HG, 64], FP32, name="vp", tag="mix")
                for c in range(KC):
                    nc.tensor.matmul(
                        vp,
                        lhsT=xT_sb[:, c, st * 128 : (st + 1) * 128],
                        rhs=wqk2v_sb[:, c, 128:320],
                        start=(c == 0),
                        stop=(c == KC - 1),
                    )
                nc.vector.tensor_add(v_sb[:, st, :, 0:64], vp, bvb_sb)

            return (480, emit)

        def qk_proj_parts(j):
            """Each weight part split into two 3-chunk halves (finer filler
            granularity); the bias add rides in the second half."""
            parts = []

            def half(wsb, w0, adds, pp_box, first):
                def emit():
                    if first:
                        pp_box[0] = mix_pool.tile(
                            [128, SB], FP32, name="pp", tag="mix"
                        )
                    pp = pp_box[0]
                    for c in (range(0, KC // 2) if first else range(KC // 2, KC)):
                        nc.tensor.matmul(
                            pp,
                            lhsT=wsb[:, c, w0 : w0 + 128],
                            rhs=xT_sb[:, c, j * SB : (j + 1) * SB],
                            start=(c == 0),
                            stop=(c == KC - 1),
                        )
                    if not first:
                        adds(pp)

                return (640, emit)

            def add01(dst, bi):
                def adds(pp):
                    nc.vector.tensor_scalar_add(
                        dst[:, j * SB : (j + 1) * SB], pp, bqk0_sb[:, bi : bi + 1]
                    )

                return adds

            def add2(pp):
                nc.vector.tensor_scalar_add(
                    qT2_sb[:, j * SB : (j + 1) * SB], pp[0:64, :], bqk1_sb[0:64, 0:1]
                )
                # k2 lands on partitions 64:128; add there, then DMA-hop the
                # bf16 rows down to partitions 0:64 (engines are lane-locked)
                ktmp = kt_pool.tile([128, SB], BF16, name="ktmp")
                nc.vector.tensor_scalar_add(
                    ktmp[64:128, :], pp[64:128, :], bqk1_sb[64:128, 1:2]
                )
                nc.gpsimd.dma_start(
                    out=kT2_sb[:, j * SB : (j + 1) * SB], in_=ktmp[64:128, :]
                )

            for wsb, w0, adds in (
                (wqk01_sb, 0, add01(qT01_sb, 0)),
                (wqk01_sb, 128, add01(kT01_sb, 1)),
                (wqk2v_sb, 0, add2),
            ):
                box = [None]
                parts.append(half(wsb, w0, adds, box, True))
                parts.append(half(wsb, w0, adds, box, False))
            return parts

        def qk_srcs(h):
            return (qT01_sb, kT01_sb, 64 * h) if h < 2 else (qT2_sb, kT2_sb, 0)

        def score_pair(j, h, t):
            """Two full key tiles t, t+1 -> one [128, 2*SB] psum -> one exp."""
            qsrc, ksrc, base = qk_srcs(h)
            et = et_sb[j % 2]
            sp = sp_pool.tile([128, 2, SB], FP32, name="sp", tag="sp")
            for u in (0, 1):
                nc.tensor.matmul(
                    sp[:, u, :],
                    lhsT=ksrc[base : base + 64, (t + u) * 128 : (t + u + 1) * 128],
                    rhs=qsrc[base : base + 64, j * SB : (j + 1) * SB],
                    start=True,
                    stop=True,
                )
            nc.scalar.activation(et[:, t : t + 2, h, :], sp, EXP, scale=SCALE)

        def score_diag(j, t, heads):
            """Diagonal tile t for a group of heads (h0+h1 batched into the
            two psum banks -> one exp with the h-adjacent et layout)."""
            et = et_sb[j % 2]
            off = 128 * t - SB * j
            n = SB - off
            sp = sp_pool.tile([128, 2, SB], FP32, name="sp", tag="sp")
            for i, h in enumerate(heads):
                qsrc, ksrc, base = qk_srcs(h)
                nc.tensor.matmul(
                    sp[:, i, 0:n],
                    lhsT=ksrc[base : base + 64, t * 128 : (t + 1) * 128],
                    rhs=qsrc[base : base + 64, j * SB + off : (j + 1) * SB],
                    start=True,
                    stop=True,
                )
            h0 = heads[0]
            nh = len(heads)
            nc.scalar.activation(
                et[:, t, h0 : h0 + nh, off:SB], sp[:, 0:nh, 0:n], EXP, scale=SCALE
            )
            for h in heads:
                nc.gpsimd.tensor_mul(
                    et[:, t, h, off : off + 128], et[:, t, h, off : off + 128], cm_sb
                )

        def block_units(j):
            """Score units for block j as (pe_ns, act_ns, fn): full pairs per
            head, then diagonal tiles (heads 0,1 batched; head 2 single)."""
            units = []
            nfull = 4 * j if causal else NT
            for h in range(HG):
                for t in range(0, nfull - 1, 2):
                    units.append((427, 1038, lambda t=t, h=h: score_pair(j, h, t)))
            if causal:
                for t in range(4 * j, tend(j)):
                    n = SB - (128 * t - SB * j)
                    units.append(
                        (int(n * 0.84), int(n * 1.67) + 185,
                         lambda t=t: score_diag(j, t, (0, 1)))
                    )
                    units.append(
                        (int(n * 0.42), int(n * 0.84) + 185,
                         lambda t=t: score_diag(j, t, (2,)))
                    )
            return units

        # per-(j,qt) ot staging: heads 0,1 share a [128, 2, 64] tile so one
        # PE transpose flips both back to [128hd, 128q]
        ot_tiles = {}

        def chain(j, qt, h):
            """attnv accumulation + normalize for one (query tile, head)."""
            et = et_sb[j % 2]
            ql = 128 * (qt - 4 * j)
            kend = qt + 1 if causal else NT
            op = op_pool.tile([128, VW], FP32, name="op")
            for t in range(kend):
                nc.tensor.matmul(
                    op,
                    lhsT=et[:, t, h, ql : ql + 128],
                    rhs=v_sb[:, t, h, :],
                    start=(t == 0),
                    stop=(t == kend - 1),
                )
            rc = rc_pool.tile([128, 1], FP32, name="rc")
            nc.vector.reciprocal(rc, op[:, 64:65])
            if h == 0:
                ot_tiles[(j, qt, 0)] = ot_pool.tile(
                    [128, 2, DK], BF16, name="ot01", tag="ot"
                )
            if h < 2:
                dst = ot_tiles[(j, qt, 0)][:, h, :]
            else:
                dst = ot_tiles[(j, qt, 2)] = ot_pool.tile(
                    [128, DK], BF16, name="ot2", tag="ot"
                )
            nc.vector.tensor_scalar_mul(dst, op[:, 0:64], rc)

        def chain_filler(j, qt, h):
            return ((qt + 1) * 27 + 90, lambda: chain(j, qt, h))

        def finish_qt(j, qt, act_copy=False, alt_pool=False):
            """Transpose the 3 heads' ot back to [hd, q] and stage for y."""
            par = j % 2
            ql = 128 * (qt - 4 * j)
            if alt_pool:
                # tail section: scores are done, spread the transpose tiles
                # across the idle sp banks to dodge mix-pool rotation stalls
                tp01 = sp_pool.tile([128, 128], BF16, name="tp01", tag="sp")
                tp2 = sp_pool.tile([64, 128], BF16, name="tp2", tag="sp")
            else:
                tp01 = mix_pool.tile([128, 128], BF16, name="tp01", tag="mix")
                tp2 = mix_pool.tile([64, 128], BF16, name="tp2", tag="mix")
            nc.tensor.transpose(tp01, ot_tiles[(j, qt, 0)], id_sb)
            nc.tensor.transpose(tp2, ot_tiles[(j, qt, 2)], id_sb)
            cp = nc.scalar.copy if act_copy else nc.vector.tensor_copy
            cp(otT01_sb[par][:, ql : ql + 128], tp01)
            cp(otT2_sb[par][:, ql : ql + 128], tp2)

        def attnv_fillers(j):
            """Fine-grained fillers for block j's attention epilogue."""
            out = []
            for qt in range(4 * j, 4 * (j + 1)):
                for h in range(HG):
                    out.append(chain_filler(j, qt, h))
                out.append((140, lambda qt=qt: finish_qt(j, qt)))
            return out

        def y_tile(j, dt, c0=0, c1=SB, qeng=None, ceng=None, pool=None):
            def emit():
                par = j % 2
                n = c1 - c0
                if pool is None:
                    yp = mix_pool.tile([128, SB], FP32, name="yp", tag="mix")
                elif pool is sp_pool:
                    yp = pool.tile([128, SB], FP32, name="yp", tag="sp")
                else:
                    yp = pool.tile([128, SB], FP32, name="yp", tag="op")
                nc.tensor.matmul(
                    yp[:, 0:n],
                    lhsT=woTP_sb[:, dt * 128 : (dt + 1) * 128],
                    rhs=otT01_sb[par][:, c0:c1],
                    start=True,
                    stop=False,
                )
                nc.tensor.matmul(
                    yp[:, 0:n],
                    lhsT=woTP_sb[0:64, D + dt * 128 : D + (dt + 1) * 128],
                    rhs=otT2_sb[par][:, c0:c1],
                    start=False,
                    stop=True,
                )
                yt = yt_pool.tile([128, SB], BF16, name="yt")
                if ceng is None:
                    nc.vector.tensor_copy(yt[:, 0:n], yp[:, 0:n])
                else:
                    ceng(yt[:, 0:n], yp[:, 0:n])
                (qeng or nc.sync).dma_start(
                    out=yT_d.ap()[
                        dt * 128 : (dt + 1) * 128, j * SB + c0 : j * SB + c1
                    ],
                    in_=yt[:, 0:n],
                )

            return (int(0.9 * (c1 - c0)), emit)

        def emit_stream(units, fillers, flush=True, after_last=True):
            """Pace filler emission by the Activation deficit: between score
            units, emit filler PE work only as fast as Act outpaces the PE on
            the scores themselves, so Act never starves. Leftovers flush at
            the stream end (their deadline) unless flush=False; with
            after_last=False no fillers are emitted after the final unit."""
            deficit = 2200  # initial credit ~= Act backlog from 2 sp bufs
            fi = 0
            spent = 0
            for i, u in enumerate(units):
                pe_c, act_c, fn = u[0], u[1], u[2]
                fn()
                deficit += max(act_c - pe_c, 0)
                if not after_last and i == len(units) - 1:
                    break
                while fi < len(fillers) and spent + fillers[fi][0] <= deficit:
                    spent += fillers[fi][0]
                    fillers[fi][1]()
                    fi += 1
            if flush:
                while fi < len(fillers):
                    fillers[fi][1]()
                    fi += 1
                return []
            return fillers[fi:]

        # ---------- main emission ----------
        last = NJ - 1
        qk_parts_late = {}
        if not causal:
            for _, f in qk_proj_parts(0):
                f()
            for j in range(NJ):
                for _, f in qk_proj_parts(j + 1) if j + 1 < NJ else []:
                    f()
                for st in range(4 * j, 4 * (j + 1)):
                    v_proj_tile(st)[1]()
            for j in range(NJ):
                fillers = []
                if j >= 1:
                    fillers += attnv_fillers(j - 1)
                    fillers += [y_tile(j - 1, dt) for dt in range(KC)]
                emit_stream(block_units(j), fillers)
            for qt in range(4 * last, 4 * (last + 1)):
                for h in range(HG):
                    chain(last, qt, h)
                finish_qt(last, qt)
            for dt in range(KC):
                y_tile(last, dt)[1]()
        else:
            for _, f in qk_proj_parts(0):
                f()
            for j in range(NJ):
                fillers = []
                if j >= 1:
                    # this block's qk2 projection (deferred from stream j-1;
                    # only head-2 scores need it, ~60% into the stream)
                    fillers += qk_parts_late[j]
                if j + 1 < NJ:
                    # next block's q0+k0 early: their DVE bias adds gate the
                    # next stream's first score tiles
                    parts = qk_proj_parts(j + 1)
                    fillers += parts[:4]
                    qk_parts_late[j + 1] = parts[4:]
                # the last block's v tiles must precede its inlined chains
                if j == last:
                    fillers += [v_proj_tile(st) for st in range(4 * j, 4 * (j + 1))]
                if j >= 1:
                    fillers += attnv_fillers(j - 1)
                if j >= 2:
                    fillers += [y_tile(j - 2, dt) for dt in range(KC)]
                if j == last:
                    fillers += [y_tile(j - 1, dt) for dt in range(KC)]
                else:
                    fillers += [v_proj_tile(st) for st in range(4 * j, 4 * (j + 1))]
                units = block_units(j)
                if j != last:
                    emit_stream(units, fillers)
                else:
                    # pace fillers against the full-tile pairs, flush whatever
                    # is left before the diag section, then interleave: chains
                    # for query tile qt run while Act processes the next
                    # tiles' diag exps
                    pre, diags = units[:-8], units[-8:]
                    left = emit_stream(pre, fillers, flush=False,
                                       after_last=False)
                    for f in left:
                        f[1]()
                    for u in range(4):
                        qt = 4 * last + u
                        for _, _, fn in diags[2 * u : 2 * u + 2]:
                            fn()
                        for h in range(HG):
                            chain(last, qt, h)
                        finish_qt(last, qt)
            # drain: output projection of the last block; copies alternate
            # DVE/Act and DMAs alternate SP/Act queues (all idle by now)
            for dt in range(KC):
                y_tile(
                    last, dt, 0, SB,
                    qeng=nc.sync if dt % 2 else nc.scalar,
                    ceng=None if dt % 2 else nc.scalar.copy,
                    pool=sp_pool if dt % 2 else None,
                )[1]()


    nc.finalize()
    return nc


_NC_CACHE: dict[bool, object] = {}


def get_nc(causal: bool):
    if causal not in _NC_CACHE:
        _NC_CACHE[causal] = build_nc(causal)
    return _NC_CACHE[causal]


def make_in_maps(x, wq, bq, wk, bk, wv, bv, wo, bo):
    """Shard full inputs into 8 per-core input maps (bf16 on the wire)."""
    import ml_dtypes

    bf16 = ml_dtypes.bfloat16
    f32 = np.float32
    p = np.arange(128)[:, None]
    c = np.arange(128)[None, :]
    cmid = np.zeros((128, 256), f32)
    cmid[:, 0:128] = p <= c
    cmid[:, 128:256] = np.eye(128, dtype=f32)
    cmid = cmid.astype(bf16)
    x = np.asarray(x, f32)
    wq, wk, wv, wo = (np.asarray(a, f32) for a in (wq, wk, wv, wo))
    bq, bk, bv = (np.asarray(a, f32) for a in (bq, bk, bv))
    in_maps = []
    for core in range(NCORES):
        b, hg = divmod(core, NH // HG)
        hs = slice(hg * HD, (hg + 1) * HD)
        wqh, wkh, wvh = wq[hs, :], wk[hs, :], wv[hs, :]
        wqk01 = np.zeros((D, 256), f32)
        wqk01[:, 0:128] = wqh[0:128].T
        wqk01[:, 128:256] = wkh[0:128].T
        wqk2v = np.zeros((D, 320), f32)
        wqk2v[:, 0:64] = wqh[128:192].T
        wqk2v[:, 64:128] = wkh[128:192].T
        wqk2v[:, 128:320] = wvh.T
        woTP = np.zeros((128, 2 * D), f32)
        woTP[:, 0:D] = wo[:, hs][:, 0:128].T
        woTP[0:64, D : 2 * D] = wo[:, hs][:, 128:192].T
        bqk0 = np.zeros((128, 2), f32)
        bqk0[:, 0] = bq[hs][0:128]
        bqk0[:, 1] = bk[hs][0:128]
        bqk1 = np.zeros((128, 2), f32)
        bqk1[0:64, 0] = bq[hs][128:192]
        bqk1[64:128, 1] = bk[hs][128:192]
        in_maps.append(
            {
                "xT": np.ascontiguousarray(x[b].T).astype(bf16),
                "wqk01": wqk01.astype(bf16),
                "wqk2v": wqk2v.astype(bf16),
                "woTP": woTP.astype(bf16),
                "bqk0": bqk0,
                "bqk1": bqk1,
                "bv": np.ascontiguousarray(bv[hs]),
                "cmid": cmid,
            }
        )
    return in_maps


def combine_outputs(results, bo):
    """Sum head-group partials per batch, transpose, add output bias."""
    y = np.empty((B, S, D), np.float32)
    ng = NH // HG
    for b in range(B):
        acc = results[b * ng]["yT"].astype(np.float32)
        for g in range(1, ng):
            acc = acc + results[b * ng + g]["yT"].astype(np.float32)
        y[b] = acc.T + np.asarray(bo, np.float32)[None, :]
    return y


def kernel(x, wq, bq, wk, bk, wv, bv, wo, bo, mask, _trace=False):
    from concourse.bass_utils import run_bass_kernel_spmd

    causal = bool(np.asarray(mask).item())
    nc = get_nc(causal)
    in_maps = make_in_maps(x, wq, bq, wk, bk, wv, bv, wo, bo)
    res = run_bass_kernel_spmd(nc, in_maps, list(range(NCORES)), trace=_trace)
    y = combine_outputs(res.results, bo)
    if _trace:
        return y, res
    return y


# revision 78
# speedup vs baseline: 1.0095x; 1.0095x over previous
"""Trainium2 Bass kernel for 12-head causal MHA (B=2, S=2048, D=768), fp32 I/O.

Sharding: 8 cores = (batch b in {0,1}) x (head-group hg in {0..3}, 3 heads each).
Each core computes, for its (b, hg):
    qT/kT = (x wq_hg^T)^T  (transposed layout, [192, S], bf16)
    v     = x wv_hg^T      ([S, 192] natural + ones column, bf16)
    causal attention with et (=exp(scores)) kept per block in SBUF, then a
    narrow-N attnv: out[128q, 65] = et[keys, q]^T @ [v | 1] accumulated over
    key tiles (bf16 runs at 1 cyc/row at ANY N, so N=65 is cheap), softmax
    denominator lands per-partition -> reciprocal + tensor_scalar_mul.
    ot[128q, 64] is PE-transposed back to [64, 128q] for the output proj.
    partial yT = wo_hg^T @ outT   ([768, S] bf16, row-parallel partial)
Host sums the 4 head-group partials per batch, transposes, adds bo.

All matmuls bf16 (1 cycle/row at any moving size). Exp on the Activation
engine with full score tiles paired into [128, 2*512] psum tiles (one exp per
pair amortizes the ~185ns Act instruction overhead). DVE handles bias adds,
normalize, and psum->sbuf drains (GPSIMD cannot touch PSUM); the all-SBUF
causal-mask multiplies run on GpSimd. Emission is software-pipelined: block
j's scores stream interleaves block j-1's attnv/output-proj plus block j+1's
projections as PE filler while Act drains exps; the last block's attnv chains
ride inside its own stream (head h's chains fill head h+1's score stream).
"""

import math
from contextlib import ExitStack

import numpy as np

import concourse.bacc as bacc
import concourse.bass as bass
import concourse.mybir as mybir
import concourse.tile as tile

FP32 = mybir.dt.float32
BF16 = mybir.dt.bfloat16

B = 2
S = 2048
D = 768
NH = 12
DK = 64
NCORES = 8
HG = 3  # heads per core
HD = HG * DK  # 192
KC = D // 128  # 6 contraction chunks of 128
SB = 512  # sequence block
NJ = S // SB  # 4
NT = S // 128  # 16 key tiles
SCALE = 1.0 / math.sqrt(DK)
VW = 65  # v tile width (64 dims + ones column for the softmax denominator)


def build_nc(causal: bool):
    nc = bacc.Bacc(trn_type="TRN2", target_bir_lowering=False, debug=False)

    xT_d = nc.declare_dram_parameter("xT", [D, S], BF16, isOutput=False)
    # packed weights (row-contiguous for fat DMA descriptors):
    # wqk01: cols 0:128 wq[0:128].T | 128:256 wk[0:128].T
    # wqk2v: cols 0:64 wq[128:192].T | 64:128 wk[128:192].T | 128:320 wv.T
    wqk01_d = nc.declare_dram_parameter("wqk01", [D, 256], BF16, isOutput=False)
    wqk2v_d = nc.declare_dram_parameter("wqk2v", [D, 320], BF16, isOutput=False)
    # wo packed: [:, 0:768] = wo[:, 0:128].T ; [0:64, 768:1536] = wo[:,128:192].T
    woTP_d = nc.declare_dram_parameter("woTP", [128, 2 * D], BF16, isOutput=False)
    bqk0_d = nc.declare_dram_parameter("bqk0", [128, 2], FP32, isOutput=False)
    bqk1_d = nc.declare_dram_parameter("bqk1", [128, 2], FP32, isOutput=False)
    bv_d = nc.declare_dram_parameter("bv", [HD], FP32, isOutput=False)
    # cols 0:128 = causal triangle mask (keep iff p <= c), 128:256 = identity
    cmid_d = nc.declare_dram_parameter("cmid", [128, 256], BF16, isOutput=False)
    yT_d = nc.declare_dram_parameter("yT", [D, S], BF16, isOutput=True)

    EXP = mybir.ActivationFunctionType.Exp

    with tile.TileContext(nc) as tc, ExitStack() as ctx:
        consts = ctx.enter_context(tc.tile_pool(name="consts", bufs=1))

        # ---- persistent SBUF tensors ----
        xT_sb = consts.tile([128, KC, S], BF16)
        wqk01_sb = consts.tile([128, KC, 256], BF16)
        wqk2v_sb = consts.tile([128, KC, 320], BF16)
        woTP_sb = consts.tile([128, 2 * D], BF16)
        bqk0_sb = consts.tile([128, 2], FP32)
        bqk1_sb = consts.tile([128, 2], FP32)
        bvb_sb = consts.tile([128, HG, 64], FP32)  # bv broadcast to partitions
        cmid_sb = consts.tile([128, 256], BF16)
        qT01_sb = consts.tile([128, S], BF16)  # q^T heads 0,1
        kT01_sb = consts.tile([128, S], BF16)
        qT2_sb = consts.tile([64, S], BF16)  # q^T head 2 (partitions 0:64)
        kT2_sb = consts.tile([64, S], BF16)  # k^T head 2 (DMA-hopped to 0:64)
        v_sb = consts.tile([128, NT, HG, VW], BF16)  # [v | ones]
        # et double-buffered by block parity (skewed pipeline overlaps blocks)
        et_sb = [consts.tile([128, NT, HG, SB], BF16, name=f"et{p}") for p in (0, 1)]
        otT01_sb = [consts.tile([128, SB], BF16, name=f"otT01_{p}") for p in (0, 1)]
        otT2_sb = [consts.tile([64, SB], BF16, name=f"otT2_{p}") for p in (0, 1)]

        cm_sb = cmid_sb[:, 0:128]
        id_sb = cmid_sb[:, 128:256]

        # ---- input DMAs: q0 weights first on SP (first matmul group),
        # x block 0 per-chunk on Pool, tiny constants on the DVE queue ----
        wqk01_r = wqk01_d.ap().rearrange("(c p) n -> p c n", p=128)
        wqk2v_r = wqk2v_d.ap().rearrange("(c p) n -> p c n", p=128)
        xT_r = xT_d.ap().rearrange("(c p) s -> p c s", p=128)
        nc.sync.dma_start(out=wqk01_sb, in_=wqk01_r)
        for c in range(KC // 2):
            nc.gpsimd.dma_start(out=xT_sb[:, c, 0:SB], in_=xT_r[:, c, 0:SB])
        nc.gpsimd.dma_start(out=bqk0_sb, in_=bqk0_d.ap())
        nc.gpsimd.dma_start(out=bqk1_sb, in_=bqk1_d.ap())
        nc.gpsimd.dma_start(out=cmid_sb, in_=cmid_d.ap())
        bv_ap = bv_d.ap()
        bvb_src = bass.AP(
            tensor=bv_ap.tensor, offset=bv_ap.offset, ap=[[0, 128], [64, HG], [1, 64]]
        )
        nc.gpsimd.dma_start(out=bvb_sb, in_=bvb_src)
        for c in range(KC // 2, KC):
            nc.scalar.dma_start(out=xT_sb[:, c, 0:SB], in_=xT_r[:, c, 0:SB])
        nc.sync.dma_start(out=wqk2v_sb, in_=wqk2v_r)
        nc.gpsimd.dma_start(out=xT_sb[:, :, SB : 2 * SB], in_=xT_r[:, :, SB : 2 * SB])
        nc.sync.dma_start(out=woTP_sb, in_=woTP_d.ap())
        for jb in range(2, NJ):
            nc.gpsimd.dma_start(
                out=xT_sb[:, :, jb * SB : (jb + 1) * SB],
                in_=xT_r[:, :, jb * SB : (jb + 1) * SB],
            )

        # preset the ones columns of v
        nc.vector.memset(v_sb[:, :, :, 64:65], 1.0)

        # ---- pools (PSUM: mix 2 + sp 4 + op 2 = 8 banks) ----
        mix_pool = ctx.enter_context(tc.tile_pool(name="mix", bufs=2, space="PSUM"))
        sp_pool = ctx.enter_context(tc.tile_pool(name="sp", bufs=2, space="PSUM"))
        op_pool = ctx.enter_context(tc.tile_pool(name="op", bufs=2, space="PSUM"))
        kt_pool = ctx.enter_context(tc.tile_pool(name="kt", bufs=3))
        rc_pool = ctx.enter_context(tc.tile_pool(name="rc", bufs=8))
        ot_pool = ctx.enter_context(tc.tile_pool(name="ot", bufs=14))
        yt_pool = ctx.enter_context(tc.tile_pool(name="yt", bufs=6))

        def tend(j):
            return 4 * (j + 1) if causal else NT

        # ---------- emission helpers ----------
        # Each filler is (pe_cost_ns, closure); streams interleave fillers
        # between score units proportionally to accumulated PE cost so the
        # Activation engine is never starved of scores nor left to lag.
        def v_proj_tile(st):
            def emit():
                vp = mix_pool.tile([128, HG, 64], FP32, name="vp", tag="mix")
                for c in range(KC):
                    nc.tensor.matmul(
                        vp,
                        lhsT=xT_sb[:, c, st * 128 : (st + 1) * 128],
                        rhs=wqk2v_sb[:, c, 128:320],
                        start=(c == 0),
                        stop=(c == KC - 1),
                    )
                nc.vector.tensor_add(v_sb[:, st, :, 0:64], vp, bvb_sb)

            return (480, emit)

        def qk_proj_parts(j):
            """Each weight part split into two 3-chunk halves (finer filler
            granularity); the bias add rides in the second half."""
            parts = []

            def half(wsb, w0, adds, pp_box, first):
                def emit():
                    if first:
                        pp_box[0] = mix_pool.tile(
                            [128, SB], FP32, name="pp", tag="mix"
                        )
                    pp = pp_box[0]
                    for c in (range(0, KC // 2) if first else range(KC // 2, KC)):
                        nc.tensor.matmul(
                            pp,
                            lhsT=wsb[:, c, w0 : w0 + 128],
                            rhs=xT_sb[:, c, j * SB : (j + 1) * SB],
                            start=(c == 0),
                            stop=(c == KC - 1),
                        )
                    if not first:
                        adds(pp)

                return (640, emit)

            def add01(dst, bi):
                def adds(pp):
                    nc.vector.tensor_scalar_add(
                        dst[:, j * SB : (j + 1) * SB], pp, bqk0_sb[:, bi : bi + 1]
                    )

                return adds

            def add2(pp):
                nc.vector.tensor_scalar_add(
                    qT2_sb[:, j * SB : (j + 1) * SB], pp[0:64, :], bqk1_sb[0:64, 0:1]
                )
                # k2 lands on partitions 64:128; add there, then DMA-hop the
                # bf16 rows down to partitions 0:64 (engines are lane-locked)
                ktmp = kt_pool.tile([128, SB], BF16, name="ktmp")
                nc.vector.tensor_scalar_add(
                    ktmp[64:128, :], pp[64:128, :], bqk1_sb[64:128, 1:2]
                )
                nc.sync.dma_start(
                    out=kT2_sb[:, j * SB : (j + 1) * SB], in_=ktmp[64:128, :]
                )

            for wsb, w0, adds in (
                (wqk01_sb, 0, add01(qT01_sb, 0)),
                (wqk01_sb, 128, add01(kT01_sb, 1)),
                (wqk2v_sb, 0, add2),
            ):
                box = [None]
                parts.append(half(wsb, w0, adds, box, True))
                parts.append(half(wsb, w0, adds, box, False))
            return parts

        def qk_srcs(h):
            return (qT01_sb, kT01_sb, 64 * h) if h < 2 else (qT2_sb, kT2_sb, 0)

        def score_pair(j, h, t):
            """Two full key tiles t, t+1 -> one [128, 2*SB] psum -> one exp."""
            qsrc, ksrc, base = qk_srcs(h)
            et = et_sb[j % 2]
            sp = sp_pool.tile([128, 2, SB], FP32, name="sp", tag="sp")
            for u in (0, 1):
                nc.tensor.matmul(
                    sp[:, u, :],
                    lhsT=ksrc[base : base + 64, (t + u) * 128 : (t + u + 1) * 128],
                    rhs=qsrc[base : base + 64, j * SB : (j + 1) * SB],
                    start=True,
                    stop=True,
                )
            nc.scalar.activation(et[:, t : t + 2, h, :], sp, EXP, scale=SCALE)

        def score_diag(j, t, heads, dve_mask=False):
            """Diagonal tile t for a group of heads (h0+h1 batched into the
            two psum banks -> one exp with the h-adjacent et layout)."""
            et = et_sb[j % 2]
            off = 128 * t - SB * j
            n = SB - off
            sp = sp_pool.tile([128, 2, SB], FP32, name="sp", tag="sp")
            for i, h in enumerate(heads):
                qsrc, ksrc, base = qk_srcs(h)
                nc.tensor.matmul(
                    sp[:, i, 0:n],
                    lhsT=ksrc[base : base + 64, t * 128 : (t + 1) * 128],
                    rhs=qsrc[base : base + 64, j * SB + off : (j + 1) * SB],
                    start=True,
                    stop=True,
                )
            h0 = heads[0]
            nh = len(heads)
            nc.scalar.activation(
                et[:, t, h0 : h0 + nh, off:SB], sp[:, 0:nh, 0:n], EXP, scale=SCALE
            )
            meng = nc.vector if dve_mask else nc.gpsimd
            for h in heads:
                meng.tensor_mul(
                    et[:, t, h, off : off + 128], et[:, t, h, off : off + 128], cm_sb
                )

        def block_units(j):
            """Score units for block j as (pe_ns, act_ns, fn): full pairs per
            head, then diagonal tiles (heads 0,1 batched; head 2 single)."""
            units = []
            nfull = 4 * j if causal else NT
            for h in range(HG):
                for t in range(0, nfull - 1, 2):
                    units.append((427, 1038, lambda t=t, h=h: score_pair(j, h, t)))
            if causal:
                dm = False  # DVE masks queue behind chain normalize work
                for t in range(4 * j, tend(j)):
                    n = SB - (128 * t - SB * j)
                    units.append(
                        (int(n * 0.84), int(n * 1.67) + 185,
                         lambda t=t: score_diag(j, t, (0, 1), dm))
                    )
                    units.append(
                        (int(n * 0.42), int(n * 0.84) + 185,
                         lambda t=t: score_diag(j, t, (2,), dm))
                    )
            return units

        # per-(j,qt) ot staging: heads 0,1 share a [128, 2, 64] tile so one
        # PE transpose flips both back to [128hd, 128q]
        ot_tiles = {}

        def chain(j, qt, h):
            """attnv accumulation + normalize for one (query tile, head)."""
            et = et_sb[j % 2]
            ql = 128 * (qt - 4 * j)
            kend = qt + 1 if causal else NT
            op = op_pool.tile([128, VW], FP32, name="op")
            for t in range(kend):
                nc.tensor.matmul(
                    op,
                    lhsT=et[:, t, h, ql : ql + 128],
                    rhs=v_sb[:, t, h, :],
                    start=(t == 0),
                    stop=(t == kend - 1),
                )
            rc = rc_pool.tile([128, 1], FP32, name="rc")
            nc.vector.reciprocal(rc, op[:, 64:65])
            if h == 0:
                ot_tiles[(j, qt, 0)] = ot_pool.tile(
                    [128, 2, DK], BF16, name="ot01", tag="ot"
                )
            if h < 2:
                dst = ot_tiles[(j, qt, 0)][:, h, :]
            else:
                dst = ot_tiles[(j, qt, 2)] = ot_pool.tile(
                    [128, DK], BF16, name="ot2", tag="ot"
                )
            nc.vector.tensor_scalar_mul(dst, op[:, 0:64], rc)

        def chain_filler(j, qt, h):
            return ((qt + 1) * 27 + 90, lambda: chain(j, qt, h))

        def finish_qt(j, qt, act_copy=False, alt_pool=False):
            """Transpose the 3 heads' ot back to [hd, q] and stage for y."""
            par = j % 2
            ql = 128 * (qt - 4 * j)
            if alt_pool:
                # tail section: scores are done, spread the transpose tiles
                # across the idle sp banks to dodge mix-pool rotation stalls
                tp01 = sp_pool.tile([128, 128], BF16, name="tp01", tag="sp")
                tp2 = sp_pool.tile([64, 128], BF16, name="tp2", tag="sp")
            else:
                tp01 = mix_pool.tile([128, 128], BF16, name="tp01", tag="mix")
                tp2 = mix_pool.tile([64, 128], BF16, name="tp2", tag="mix")
            nc.tensor.transpose(tp01, ot_tiles[(j, qt, 0)], id_sb)
            nc.tensor.transpose(tp2, ot_tiles[(j, qt, 2)], id_sb)
            cp = nc.scalar.copy if act_copy else nc.vector.tensor_copy
            cp(otT01_sb[par][:, ql : ql + 128], tp01)
            cp(otT2_sb[par][:, ql : ql + 128], tp2)

        def attnv_fillers(j):
            """Fine-grained fillers for block j's attention epilogue."""
            out = []
            for qt in range(4 * j, 4 * (j + 1)):
                for h in range(HG):
                    out.append(chain_filler(j, qt, h))
                out.append((140, lambda qt=qt: finish_qt(j, qt)))
            return out

        def y_tile(j, dt, c0=0, c1=SB, qeng=None, ceng=None, pool=None):
            def emit():
                par = j % 2
                n = c1 - c0
                if pool is None:
                    yp = mix_pool.tile([128, SB], FP32, name="yp", tag="mix")
                elif pool is sp_pool:
                    yp = pool.tile([128, SB], FP32, name="yp", tag="sp")
                else:
                    yp = pool.tile([128, SB], FP32, name="yp", tag="op")
                nc.tensor.matmul(
                    yp[:, 0:n],
                    lhsT=woTP_sb[:, dt * 128 : (dt + 1) * 128],
                    rhs=otT01_sb[par][:, c0:c1],
                    start=True,
                    stop=False,
                )
                nc.tensor.matmul(
                    yp[:, 0:n],
                    lhsT=woTP_sb[0:64, D + dt * 128 : D + (dt + 1) * 128],
                    rhs=otT2_sb[par][:, c0:c1],
                    start=False,
                    stop=True,
                )
                yt = yt_pool.tile([128, SB], BF16, name="yt")
                if ceng is None:
                    nc.vector.tensor_copy(yt[:, 0:n], yp[:, 0:n])
                else:
                    ceng(yt[:, 0:n], yp[:, 0:n])
                (qeng or nc.sync).dma_start(
                    out=yT_d.ap()[
                        dt * 128 : (dt + 1) * 128, j * SB + c0 : j * SB + c1
                    ],
                    in_=yt[:, 0:n],
                )

            return (int(0.9 * (c1 - c0)), emit)

        def emit_stream(units, fillers, flush=True, after_last=True):
            """Pace filler emission by the Activation deficit: between score
            units, emit filler PE work only as fast as Act outpaces the PE on
            the scores themselves, so Act never starves. Leftovers flush at
            the stream end (their deadline) unless flush=False; with
            after_last=False no fillers are emitted after the final unit."""
            deficit = 2200  # initial credit ~= Act backlog from 2 sp bufs
            fi = 0
            spent = 0
            for i, u in enumerate(units):
                pe_c, act_c, fn = u[0], u[1], u[2]
                fn()
                deficit += max(act_c - pe_c, 0)
                if not after_last and i == len(units) - 1:
                    break
                while fi < len(fillers) and spent + fillers[fi][0] <= deficit:
                    spent += fillers[fi][0]
                    fillers[fi][1]()
                    fi += 1
            if flush:
                while fi < len(fillers):
                    fillers[fi][1]()
                    fi += 1
                return []
            return fillers[fi:]

        # ---------- main emission ----------
        last = NJ - 1
        qk_parts_late = {}
        if not causal:
            for _, f in qk_proj_parts(0):
                f()
            for j in range(NJ):
                for _, f in qk_proj_parts(j + 1) if j + 1 < NJ else []:
                    f()
                for st in range(4 * j, 4 * (j + 1)):
                    v_proj_tile(st)[1]()
            for j in range(NJ):
                fillers = []
                if j >= 1:
                    fillers += attnv_fillers(j - 1)
                    fillers += [y_tile(j - 1, dt) for dt in range(KC)]
                emit_stream(block_units(j), fillers)
            for qt in range(4 * last, 4 * (last + 1)):
                for h in range(HG):
                    chain(last, qt, h)
                finish_qt(last, qt)
            for dt in range(KC):
                y_tile(last, dt)[1]()
        else:
            for _, f in qk_proj_parts(0):
                f()
            for j in range(NJ):
                fillers = []
                if j >= 1:
                    # this block's qk2 projection (deferred from stream j-1;
                    # only head-2 scores need it, ~60% into the stream)
                    fillers += qk_parts_late[j]
                if j + 1 < NJ:
                    # next block's q0+k0 early: their DVE bias adds gate the
                    # next stream's first score tiles
                    parts = qk_proj_parts(j + 1)
                    fillers += parts[:4]
                    qk_parts_late[j + 1] = parts[4:]
                # the last block's v tiles must precede its inlined chains
                if j == last:
                    fillers += [v_proj_tile(st) for st in range(4 * j, 4 * (j + 1))]
                if j >= 1:
                    fillers += attnv_fillers(j - 1)
                if j >= 2:
                    fillers += [y_tile(j - 2, dt) for dt in range(KC)]
                if j == last:
                    fillers += [y_tile(j - 1, dt) for dt in range(KC)]
                else:
                    fillers += [v_proj_tile(st) for st in range(4 * j, 4 * (j + 1))]
                units = block_units(j)
                if j != last:
                    emit_stream(units, fillers)
                else:
                    # pace fillers against the full-tile pairs, flush whatever
                    # is left before the diag section, then interleave: chains
                    # for query tile qt run while Act processes the next
                    # tiles' diag exps
                    pre, diags = units[:-8], units[-8:]
                    left = emit_stream(pre, fillers, flush=False,
                                       after_last=False)
                    for f in left:
                        f[1]()
                    for u in range(4):
                        qt = 4 * last + u
                        for _, _, fn in diags[2 * u : 2 * u + 2]:
                            fn()
                        for h in range(HG):
                            chain(last, qt, h)
                        finish_qt(last, qt)
            # drain: output projection of the last block; copies alternate
            # DVE/Act and DMAs alternate SP/Act queues (all idle by now)
            for dt in range(KC):
                y_tile(
                    last, dt, 0, SB,
                    qeng=nc.sync if dt % 2 else nc.scalar,
                    ceng=None if dt % 2 else nc.scalar.copy,
                    pool=sp_pool if dt % 2 else None,
                )[1]()


    nc.finalize()
    return nc


_NC_CACHE: dict[bool, object] = {}


def get_nc(causal: bool):
    if causal not in _NC_CACHE:
        _NC_CACHE[causal] = build_nc(causal)
    return _NC_CACHE[causal]


def make_in_maps(x, wq, bq, wk, bk, wv, bv, wo, bo):
    """Shard full inputs into 8 per-core input maps (bf16 on the wire)."""
    import ml_dtypes

    bf16 = ml_dtypes.bfloat16
    f32 = np.float32
    p = np.arange(128)[:, None]
    c = np.arange(128)[None, :]
    cmid = np.zeros((128, 256), f32)
    cmid[:, 0:128] = p <= c
    cmid[:, 128:256] = np.eye(128, dtype=f32)
    cmid = cmid.astype(bf16)
    x = np.asarray(x, f32)
    wq, wk, wv, wo = (np.asarray(a, f32) for a in (wq, wk, wv, wo))
    bq, bk, bv = (np.asarray(a, f32) for a in (bq, bk, bv))
    in_maps = []
    for core in range(NCORES):
        b, hg = divmod(core, NH // HG)
        hs = slice(hg * HD, (hg + 1) * HD)
        wqh, wkh, wvh = wq[hs, :], wk[hs, :], wv[hs, :]
        wqk01 = np.zeros((D, 256), f32)
        wqk01[:, 0:128] = wqh[0:128].T
        wqk01[:, 128:256] = wkh[0:128].T
        wqk2v = np.zeros((D, 320), f32)
        wqk2v[:, 0:64] = wqh[128:192].T
        wqk2v[:, 64:128] = wkh[128:192].T
        wqk2v[:, 128:320] = wvh.T
        woTP = np.zeros((128, 2 * D), f32)
        woTP[:, 0:D] = wo[:, hs][:, 0:128].T
        woTP[0:64, D : 2 * D] = wo[:, hs][:, 128:192].T
        bqk0 = np.zeros((128, 2), f32)
        bqk0[:, 0] = bq[hs][0:128]
        bqk0[:, 1] = bk[hs][0:128]
        bqk1 = np.zeros((128, 2), f32)
        bqk1[0:64, 0] = bq[hs][128:192]
        bqk1[64:128, 1] = bk[hs][128:192]
        in_maps.append(
            {
                "xT": np.ascontiguousarray(x[b].T).astype(bf16),
                "wqk01": wqk01.astype(bf16),
                "wqk2v": wqk2v.astype(bf16),
                "woTP": woTP.astype(bf16),
                "bqk0": bqk0,
                "bqk1": bqk1,
                "bv": np.ascontiguousarray(bv[hs]),
                "cmid": cmid,
            }
        )
    return in_maps


def combine_outputs(results, bo):
    """Sum head-group partials per batch, transpose, add output bias."""
    y = np.empty((B, S, D), np.float32)
    ng = NH // HG
    for b in range(B):
        acc = results[b * ng]["yT"].astype(np.float32)
        for g in range(1, ng):
            acc = acc + results[b * ng + g]["yT"].astype(np.float32)
        y[b] = acc.T + np.asarray(bo, np.float32)[None, :]
    return y


def kernel(x, wq, bq, wk, bk, wv, bv, wo, bo, mask, _trace=False):
    from concourse.bass_utils import run_bass_kernel_spmd

    causal = bool(np.asarray(mask).item())
    nc = get_nc(causal)
    in_maps = make_in_maps(x, wq, bq, wk, bk, wv, bv, wo, bo)
    res = run_bass_kernel_spmd(nc, in_maps, list(range(NCORES)), trace=_trace)
    y = combine_outputs(res.results, bo)
    if _trace:
        return y, res
    return y
